# revision 12
# baseline (speedup 1.0000x reference)
"""MultiHeadAttention Trainium2 Bass kernel, 8-core SPMD.

Problem: B=4, S=2048, E=2048, H=16, Dh=128; reshape-based (not transposed)
head split:  q = (x@Wq).reshape(B,H,S,Dh) etc., softmax over the QUERY axis,
out = attn.reshape(B,S,E).

Key structure: flattening (B,S) rows, row-block gp (128 rows) of x@W is
exactly head pair gp=(b,h): Qh = Y[128gp:128gp+128,:].reshape(2048,128).
Each of the 8 cores handles 8 consecutive pairs -> core c gets contiguous
x rows [1024c:1024c+1024) and produces the same output rows. No collectives.

Per-core internal q/k index permutation (order-free since softmax reduces
over q): f = j*128 + s  <->  q = 16s + j. With that permutation:
  QT/KT [d, f]  = the j-th 128-col block of (Xblk @ W)^T, stored contiguous
  Vh block kj   = rows of Yv = Xblk@Wv in natural [s, e] layout, e-block kj
  out block     = per-128-col transpose of attnT.

Dtypes: fp32r (TF32-like, 1cy/row at N>=256) for projections + scores;
bf16 for softmax weights + attn; fp32 PSUM/softmax stats throughout.
Measured end-to-end numerics (numpy sim): rel L2 ~ 3.5e-3 vs fp32 ref.
"""

import numpy as np
from contextlib import ExitStack

import concourse.bass as bass
import concourse.tile as tile
from concourse import bacc, mybir
from concourse.bass import ds, ts
from concourse.bass_utils import run_bass_kernel_spmd
from concourse.masks import make_identity

F32 = mybir.dt.float32
F32R = mybir.dt.float32r
BF16 = mybir.dt.bfloat16
AX = mybir.AxisListType.X
EXP = mybir.ActivationFunctionType.Exp

P = 128
NPAIR = 8          # (b,h) pairs per core
GRP = 4            # pairs per phase group
NGRP = NPAIR // GRP
NJ = 16            # 128-blocks in E / contraction
G = 6              # max kj per attn accumulation group (groups 6,6,4)
GROUP_START = {5: 0, 11: 6, 15: 12}   # kj at group end -> group start
SCALE = 1.0 / np.sqrt(128.0)

_cache = {}


def _emit(nc, tc, ctx, xl, wq, wk, wv, idr, out, reps=1):
    sb = ctx.enter_context
    pIN = sb(tc.tile_pool(name="pin", bufs=1))
    pXT = sb(tc.tile_pool(name="pxt", bufs=1))
    pW = sb(tc.tile_pool(name="pw", bufs=2))
    pSTG = sb(tc.tile_pool(name="pstg", bufs=2))
    pYV = sb(tc.tile_pool(name="pyv", bufs=4))
    pQK = sb(tc.tile_pool(name="pqk", bufs=2))
    pSOFT = sb(tc.tile_pool(name="psoft", bufs=G + 1))
    pVS = sb(tc.tile_pool(name="pvs", bufs=G + 1))
    pACC = sb(tc.tile_pool(name="pacc", bufs=2))
    pST = sb(tc.tile_pool(name="pst", bufs=8))
    pCONST = sb(tc.tile_pool(name="pconst", bufs=1))
    psSC = sb(tc.tile_pool(name="pssc", bufs=2, space="PSUM"))   # [128,1024] x2 = 4 banks
    psAT = sb(tc.tile_pool(name="psat", bufs=1, space="PSUM"))   # [128,1024]    = 2 banks
    psMX = sb(tc.tile_pool(name="psmx", bufs=2, space="PSUM"))   # [128,512] x2  = 2 banks
    dram = sb(tc.tile_pool(name="dram", bufs=1, space="DRAM"))

    qsp = dram.tile([NPAIR, NJ, P, P], F32R, tag="qsp")
    ksp = dram.tile([NPAIR, NJ, P, P], F32R, tag="ksp")

    ident = pCONST.tile([P, P], F32, tag="ident")
    make_identity(nc, ident[:])
    identr = pCONST.tile([P, P], F32R, tag="identr")
    nc.sync.dma_start(identr[:], idr)
    ident_r = identr[:]

    yv_tiles = {}

    def phase_a(grp):
        """Transpose the group's x blocks into XTg [P, kb, pair, s] (f32r)."""
        xtg = pXT.tile([P, NJ, GRP, P], F32R, tag="xtg")
        for pi in range(GRP):
            gp = grp * GRP + pi
            xt = pIN.tile([P, NJ * P], F32R, tag="xt")
            nc.sync.dma_start(xt[:], xl[ds(gp * P, P), :])
            for jj in range(4):
                pt = psMX.tile([P, 512], F32, tag="mx")
                for i in range(4):
                    j = jj * 4 + i
                    nc.tensor.transpose(
                        pt[:, ds(i * P, P)].bitcast(F32R), xt[:, ds(j * P, P)], ident_r
                    )
                nc.vector.tensor_copy(
                    xtg[:, ts(jj, 4), pi, :], pt[:].rearrange("p (a b) -> p a b", a=4)
                )
        return xtg

    def phase_b(grp, xtg):
        """Projections for the group's 4 pairs; spill QT/KT, keep YV in SBUF."""
        for wname, wd, sp in (("q", wq, qsp), ("k", wk, ksp)):
            for j in range(NJ):
                wt = pW.tile([P, NJ, P], F32R, tag="wqk")
                nc.sync.dma_start(
                    wt[:], wd[:, ds(j * P, P)].rearrange("(kb p) m -> p kb m", p=P)
                )
                ps = psMX.tile([P, 512], F32, tag="mx")
                for kb in range(NJ):
                    nc.tensor.matmul(
                        ps[:], wt[:, kb], xtg[:, kb], start=(kb == 0), stop=(kb == NJ - 1)
                    )
                stg = pSTG.tile([P, GRP, P], F32R, tag="stg")
                nc.vector.tensor_copy(stg[:], ps[:].rearrange("p (g s) -> p g s", g=GRP))
                nc.sync.dma_start(
                    sp[ds(grp * GRP, GRP), j].rearrange("g d s -> d g s"), stg[:]
                )
        for pi in range(GRP):
            yv_tiles[grp * GRP + pi] = pYV.tile(
                [P, NJ * P], F32, tag="yv", name=f"yv{grp * GRP + pi}"
            )
        for ec in range(8):
            wvt = pW.tile([P, NJ, 256], F32R, tag="wv")
            nc.sync.dma_start(
                wvt[:], wv[:, ds(ec * 256, 256)].rearrange("(kb p) n -> p kb n", p=P)
            )
            for pi in range(GRP):
                gp = grp * GRP + pi
                ps = psMX.tile([P, 512], F32, tag="mx")
                for kb in range(NJ):
                    nc.tensor.matmul(
                        ps[:, :256], xtg[:, kb, pi], wvt[:, kb],
                        start=(kb == 0), stop=(kb == NJ - 1),
                    )
                nc.vector.tensor_copy(yv_tiles[gp][:, ds(ec * 256, 256)], ps[:, :256])

    def phase_c(gp):
        """Scores + softmax-over-q + attn + output for one pair."""
        qt = pQK.tile([P, NJ, P], F32R, tag="qt")
        nc.sync.dma_start(qt[:], qsp[gp].rearrange("j d s -> d j s"))
        kt = pQK.tile([P, NJ, P], F32R, tag="kt")
        nc.sync.dma_start(kt[:], ksp[gp].rearrange("j d s -> d j s"))
        yv = yv_tiles.pop(gp)
        acc = pACC.tile([P, NJ * P], F32, tag="acc")
        softs, vss = {}, {}
        for kj in range(NJ):
            soft = pSOFT.tile([P, 2048], BF16, tag="soft")
            pss, nms = [], []
            for h in range(2):
                ps = psSC.tile([P, 1024], F32, tag="sc")
                for c in range(2):
                    nc.tensor.matmul(
                        ps[:, ds(c * 512, 512)], kt[:, kj], qt[:, ts(h * 2 + c, 4)],
                        start=True, stop=True,
                    )
                nm = pST.tile([P, 1], F32, tag="nm")
                nc.vector.reduce_max(nm[:], ps[:], axis=AX, negate=True)
                pss.append(ps)
                nms.append(nm)
            ng = pST.tile([P, 1], F32, tag="ng")
            nc.vector.tensor_tensor(ng[:], nms[0][:], nms[1][:], mybir.AluOpType.min)
            ngs = pST.tile([P, 1], F32, tag="ngs")
            nc.vector.tensor_scalar_mul(ngs[:], ng[:], SCALE)
            lsum = pST.tile([P, 2], F32, tag="ls")
            for h in range(2):
                nc.scalar.activation(
                    soft[:, ds(h * 1024, 1024)], pss[h][:], EXP,
                    bias=ngs[:], scale=SCALE, accum_out=lsum[:, ds(h, 1)],
                )
            lt = pST.tile([P, 1], F32, tag="lt")
            nc.vector.reduce_sum(lt[:], lsum[:], axis=AX)
            rcp = pST.tile([P, 1], F32, tag="rcp")
            nc.vector.reciprocal(rcp[:], lt[:])
            vs = pVS.tile([P, P], BF16, tag="vs")
            nc.vector.tensor_scalar_mul(vs[:], yv[:, ts(kj, P)], rcp[:])
            softs[kj], vss[kj] = soft, vs
            if kj in GROUP_START:
                g0 = GROUP_START[kj]
                glen = kj - g0 + 1
                for h in range(2):
                    pa = psAT.tile([P, 1024], F32, tag="at")
                    for c in range(2):
                        for i in range(glen):
                            k2 = g0 + i
                            nc.tensor.matmul(
                                pa[:, ds(c * 512, 512)], vss[k2][:],
                                softs[k2][:, ds(h * 1024 + c * 512, 512)],
                                start=(i == 0), stop=(i == glen - 1),
                            )
                    if g0 == 0:
                        nc.vector.tensor_copy(acc[:, ds(h * 1024, 1024)], pa[:])
                    else:
                        nc.vector.tensor_add(
                            acc[:, ds(h * 1024, 1024)], acc[:, ds(h * 1024, 1024)], pa[:]
                        )
        for jj in range(4):
            pt = psMX.tile([P, 512], F32, tag="mx")
            for i in range(4):
                c = jj * 4 + i
                nc.tensor.transpose(pt[:, ds(i * P, P)], acc[:, ds(c * P, P)], ident[:])
            nc.scalar.copy(acc[:, ds(jj * 512, 512)], pt[:])
        nc.sync.dma_start(out[ds(gp * P, P), :], acc[:])

    for _rep in range(reps):
        xtg = phase_a(0)
        phase_b(0, xtg)
        for pi in range(GRP):
            phase_c(pi)
        xtg = phase_a(1)
        phase_b(1, xtg)
        for pi in range(GRP):
            phase_c(GRP + pi)


def build(reps=1):
    if ("nc", reps) in _cache:
        return _cache[("nc", reps)]
    nc = bacc.Bacc("TRN2", target_bir_lowering=False, debug=False)
    xl = nc.dram_tensor("xl", [NPAIR * P, 2048], F32R, kind="ExternalInput").ap()
    wq = nc.dram_tensor("wq", [2048, 2048], F32R, kind="ExternalInput").ap()
    wk = nc.dram_tensor("wk", [2048, 2048], F32R, kind="ExternalInput").ap()
    wv = nc.dram_tensor("wv", [2048, 2048], F32R, kind="ExternalInput").ap()
    idr = nc.dram_tensor("idr", [P, P], F32R, kind="ExternalInput").ap()
    out = nc.dram_tensor("out", [NPAIR * P, 2048], F32, kind="ExternalOutput").ap()
    with tile.TileContext(nc) as tc:
        with ExitStack() as ctx:
            _emit(nc, tc, ctx, xl, wq, wk, wv, idr, out, reps=reps)
    nc.compile()
    _cache[("nc", reps)] = nc
    return nc


def kernel(x, w_query, w_key, w_value, _want_trace=False):
    x = np.ascontiguousarray(np.asarray(x, np.float32))
    wq = np.ascontiguousarray(np.asarray(w_query, np.float32))
    wk = np.ascontiguousarray(np.asarray(w_key, np.float32))
    wv = np.ascontiguousarray(np.asarray(w_value, np.float32))
    B, S, E = x.shape
    xf = x.reshape(B * S, E)
    nc = build()
    rows = NPAIR * P
    in_maps = [
        dict(xl=np.ascontiguousarray(xf[c * rows:(c + 1) * rows]),
             wq=wq, wk=wk, wv=wv, idr=np.eye(P, dtype=np.float32))
        for c in range(8)
    ]
    res = run_bass_kernel_spmd(nc, in_maps, core_ids=list(range(8)),
                               trace=_want_trace)
    outf = np.concatenate([r["out"] for r in res.results], axis=0)
    if _want_trace:
        kernel.last_result = res
    return outf.reshape(B, S, E)
